# revision 5
# baseline (speedup 1.0000x reference)
"""3-layer GCN (GCNConv x3) on Trainium via jax/XLA-neuron, sharded over 8 cores.

Strategy (graph/data parallel per sharding hint):
- Nodes partitioned into 8 contiguous shards (dst ownership).
- Edges bucketed by dst shard on host; each core aggregates messages for its
  own dst nodes only, gathering h[src] from a replicated activation table.
- Small weight matrices replicated; per-layer activation tables are
  re-assembled (allgathered) between layers.
- Layer algebra: propagation always runs in the 64-wide representation:
    L1: t1 = x @ W1;  o1 = relu(A_hat t1 + b1)
    L2: p2 = A_hat o1; o2 = relu(p2 @ W2 + b2)
    L3: t3 = o2 @ W3; out = A_hat t3 + b3
  (A_hat commutes with the feature-space matmul.)
"""
import numpy as np

N = 100000
N_CORES = 8

_jit_cache = {}


def _get_fns():
    if 'fns' in _jit_cache:
        return _jit_cache['fns']
    import jax
    import jax.numpy as jnp

    devs = jax.devices()[:N_CORES]

    def dense(x, W):
        return x @ W

    # per-core aggregation: messages for edges whose dst lives on this core,
    # processed in fixed-size chunks (neuronx-cc 16-bit limits on huge gathers)
    def agg_chunk(table, src, dst_local, norm, acc):
        msg = table[src] * norm[:, None]
        return acc + jax.ops.segment_sum(msg, dst_local, num_segments=acc.shape[0])

    agg_chunk_j = jax.jit(agg_chunk)

    def finish_relu(acc, bias):
        return jax.nn.relu(acc + bias)

    def finish_lin(acc, bias):
        return acc + bias

    fin_relu_j = jax.jit(finish_relu)
    fin_lin_j = jax.jit(finish_lin)

    def agg_full(table, src, dst_local, norm, n_local, bias, do_relu, zeros):
        acc = zeros
        K = src.shape[0]
        for k in range(K):
            acc = agg_chunk_j(table, src[k], dst_local[k], norm[k], acc)
        return (fin_relu_j if do_relu else fin_lin_j)(acc, bias)

    agg_relu = lambda t, s, d, n, nl, b, z: agg_full(t, s, d, n, nl, b, True, z)
    agg_lin = lambda t, s, d, n, nl, b, z: agg_full(t, s, d, n, nl, b, False, z)
    dense_j = jax.jit(dense)
    relu_dense_j = jax.jit(lambda x, W, b, W2: jax.nn.relu(x @ W + b) @ W2)
    fns = (jax, jnp, devs, dense_j, agg_relu, agg_lin, relu_dense_j)
    _jit_cache['fns'] = fns
    return fns


def kernel(x, edge_index, W1, b1, W2, b2, W3, b3):
    jax, jnp, devs, dense_j, agg_relu, agg_lin, relu_dense_j = _get_fns()

    x = np.asarray(x)
    edge_index = np.asarray(edge_index)
    W1, b1, W2, b2, W3, b3 = (np.asarray(a) for a in (W1, b1, W2, b2, W3, b3))

    # ---- host: degrees / norms (same A_hat for all layers) ----
    src = edge_index[0].astype(np.int64)
    dst = edge_index[1].astype(np.int64)
    loop = np.arange(N, dtype=np.int64)
    src_f = np.concatenate([src, loop])
    dst_f = np.concatenate([dst, loop])
    deg = np.bincount(dst_f, minlength=N).astype(np.float32)
    dinv = np.where(deg > 0, 1.0 / np.sqrt(deg), 0.0).astype(np.float32)
    norm = (dinv[src_f] * dinv[dst_f]).astype(np.float32)

    # ---- host: shard edges by dst ownership ----
    shard = N // N_CORES                      # 12500
    owner = dst_f // shard
    order = np.argsort(owner, kind='stable')
    src_s, dst_s, norm_s = src_f[order], dst_f[order], norm[order]
    counts = np.bincount(owner, minlength=N_CORES)
    offs = np.concatenate([[0], np.cumsum(counts)])

    CH = 4096
    K = int(np.ceil(counts.max() / CH))
    per_core = []
    for c in range(N_CORES):
        a, b = offs[c], offs[c + 1]
        n_e = b - a
        pad = K * CH - n_e
        # padded edges: gather row 0, scatter into dummy row (shard index n/a ->
        # use dst 0 with norm 0 so contribution is zero)
        s = np.concatenate([src_s[a:b], np.zeros(pad, np.int64)]).astype(np.int32)
        d = np.concatenate([dst_s[a:b] - c * shard, np.zeros(pad, np.int64)]).astype(np.int32)
        nr = np.concatenate([norm_s[a:b], np.zeros(pad, np.float32)]).astype(np.float32)
        per_core.append((s.reshape(K, CH), d.reshape(K, CH), nr.reshape(K, CH)))

    # ---- device pipeline ----
    def put(c, arr):
        return jax.device_put(arr, devs[c])

    edata = []
    for c in range(N_CORES):
        s, d, nrm = per_core[c]
        edata.append((put(c, s), put(c, d), put(c, nrm)))

    W1d = [put(c, W1) for c in range(N_CORES)]
    W2d = [put(c, W2) for c in range(N_CORES)]
    W3d = [put(c, W3) for c in range(N_CORES)]
    b1d = [put(c, b1) for c in range(N_CORES)]
    b2d = [put(c, b2) for c in range(N_CORES)]
    b3d = [put(c, b3) for c in range(N_CORES)]

    # L1 dense: each core computes t1 for its own node shard, then allgather.
    xs = [put(c, x[c * shard:(c + 1) * shard]) for c in range(N_CORES)]
    t1_sh = [dense_j(xs[c], W1d[c]) for c in range(N_CORES)]
    t1 = np.concatenate([np.asarray(t) for t in t1_sh], axis=0)

    # L1 aggregation (+b1, relu) per core over its dst shard
    t1_rep = [put(c, t1) for c in range(N_CORES)]
    zer = [put(c, np.zeros((shard, 64), np.float32)) for c in range(N_CORES)]
    o1_sh = [agg_relu(t1_rep[c], *edata[c], shard, b1d[c], zer[c]) for c in range(N_CORES)]
    o1 = np.concatenate([np.asarray(t) for t in o1_sh], axis=0)

    # L2 aggregation (propagate-first), then dense relu(p2 W2 + b2) W3
    o1_rep = [put(c, o1) for c in range(N_CORES)]
    zer = [put(c, np.zeros((shard, 64), np.float32)) for c in range(N_CORES)]
    p2_sh = [agg_lin(o1_rep[c], *edata[c], shard, jnp.zeros(64, jnp.float32), zer[c]) for c in range(N_CORES)]
    t3_sh = [relu_dense_j(p2_sh[c], W2d[c], b2d[c], W3d[c]) for c in range(N_CORES)]
    t3 = np.concatenate([np.asarray(t) for t in t3_sh], axis=0)

    # L3 aggregation + b3 (no relu)
    t3_rep = [put(c, t3) for c in range(N_CORES)]
    zer = [put(c, np.zeros((shard, 64), np.float32)) for c in range(N_CORES)]
    out_sh = [agg_lin(t3_rep[c], *edata[c], shard, b3d[c], zer[c]) for c in range(N_CORES)]
    out = np.concatenate([np.asarray(t) for t in out_sh], axis=0)
    return out.astype(np.float32)
